# revision 19
# baseline (speedup 1.0000x reference)
"""Cross-attention + parallel-FF block on 8 Trainium2 cores (Bass/Tile).

Sharding: rows of x (sequence-parallel). Each core processes 512 of the 4096
query rows: LN, q-projection (all 8 heads), full attention over the shared
(multi-query) K/V, output projection and the full FF for its rows. K/V are
computed from the full context on every core (replicated, no collectives).

Engine budget (cost-model): exp is Activation-only (~137us) so every other
elementwise op is placed on DVE/Pool; the FF matmuls stay bf16 (fp8 would
blow the 2e-2 error budget) while the attention cluster (k/v proj, sim,
attn@v) runs fp8e4m3 with DoubleRow packing — attention output is a
near-uniform average over 4096 context rows, so fp8 noise washes out.
ff1 tiles are interleaved into the ctx phase and the attention phase so the
PE stays fed while DVE (layernorm) and Act (exp) run.

Scales: Wq carries gamma and dh^-0.5; Wkv carries ctx_gamma and a x16 boost
(keeps fp8 weights out of subnormals). sim_psum = (16q)(16k) = 256 sim, so
exp uses scale=1/256; exp output is 8*exp (bias=ln 8) to sit in e4m3 range;
the "ones" column of V holds 16 so numerator (16v * 8exp) / denominator
(16 * 8exp) is exact.
"""

import math

import numpy as np
import ml_dtypes

import concourse.bass as bass
import concourse.tile as tile
from concourse import bacc, mybir
from concourse.bass import ts
from concourse.masks import make_identity

BF16 = mybir.dt.bfloat16
F32 = mybir.dt.float32
FP8 = mybir.dt.float8e4

N_CORES = 8
N = 4096            # query rows (total)
NS = N // N_CORES   # rows per core = 512
D = 1024            # model dim
J = 4096            # context rows
H = 8               # heads
DH = 64             # head dim
INNER = H * DH      # 512
FF = 4096           # ff_inner
EPS = 1e-5

NT = NS // 128      # 4   query-row tiles per core
JT = J // 128       # 32  context-row tiles
DC = D // 128       # 8   feature chunks
FT = FF // 128      # 32  ff tiles (per a/gate half)

KV_SCALE = 16.0     # host-side Wkv multiplier (fp8 subnormal avoidance)
EXP_SCALE = 1.0 / (KV_SCALE * KV_SCALE)  # sim psum carries 16q * 16k
EXP_BIAS = math.log(8.0)                 # et = 8 * exp(sim)
FF1_IN_PH3 = 12     # ff1 tiles interleaved into the ctx phase (rest: attn)

DR = mybir.MatmulPerfMode.DoubleRow


def _ln_stats(nc, pool, x_tile, eps_ap):
    """bn_stats layer norm stats: returns (r, nmr) f32 [128,1] tiles."""
    stats = pool.tile([128, 2, 6], F32, tag="stats")
    mv = pool.tile([128, 2], F32, tag="mv")
    sq = pool.tile([128, 1], F32, tag="sq")
    r = pool.tile([128, 1], F32, tag="r")
    nmr = pool.tile([128, 1], F32, tag="nmr")
    xv = x_tile[:].rearrange("p (a b) -> p a b", b=512)
    nc.vector.bn_stats(stats[:, 0, :], xv[:, 0, :])
    nc.vector.bn_stats(stats[:, 1, :], xv[:, 1, :])
    nc.vector.bn_aggr(mv[:], stats[:])
    nc.scalar.activation(sq[:], mv[:, 1:2], mybir.ActivationFunctionType.Sqrt,
                         bias=eps_ap, scale=1.0)
    nc.vector.reciprocal(r[:], sq[:])
    nc.vector.scalar_tensor_tensor(nmr[:], mv[:, 0:1], -1.0, r[:],
                                   op0=mybir.AluOpType.mult,
                                   op1=mybir.AluOpType.mult)
    return r, nmr


def build(reps=1):
    nc = bacc.Bacc("TRN2", target_bir_lowering=False, debug=False,
                   num_devices=N_CORES)

    xs_d = nc.dram_tensor("xs", [NS, D], BF16, kind="ExternalInput")
    ctx_d = nc.dram_tensor("ctx", [J, D], BF16, kind="ExternalInput")
    wq_d = nc.dram_tensor("wq", [D, INNER], BF16, kind="ExternalInput")
    wkv_d = nc.dram_tensor("wkv", [D, 2 * DH], FP8, kind="ExternalInput")
    wout_d = nc.dram_tensor("wout", [INNER, D], BF16, kind="ExternalInput")
    wff1_d = nc.dram_tensor("wff1", [2 * FT, 128, DC, 128], BF16,
                            kind="ExternalInput")
    wff2_d = nc.dram_tensor("wff2", [FF, D], BF16, kind="ExternalInput")
    out_d = nc.dram_tensor("out", [NS, D], F32, kind="ExternalOutput")

    with tile.TileContext(nc) as tc:
        with (
            tc.tile_pool(name="const", bufs=1) as constp,
            tc.tile_pool(name="weights", bufs=1) as wp,
            tc.tile_pool(name="resident", bufs=1) as rp,
            tc.tile_pool(name="work", bufs=3) as work,
            tc.tile_pool(name="expt", bufs=6) as expp,
            tc.tile_pool(name="small", bufs=8) as small,
            tc.tile_pool(name="wstream", bufs=6) as ws,
        ):
            ident = constp.tile([128, 128], BF16)
            make_identity(nc, ident[:])
            eps_t = constp.tile([128, 1], F32)
            nc.gpsimd.memset(eps_t[:], EPS)
            expb_t = constp.tile([128, 1], F32)
            nc.gpsimd.memset(expb_t[:], EXP_BIAS)

            wq_sb = wp.tile([128, DC, INNER], BF16)
            nc.scalar.dma_start(wq_sb[:], wq_d.ap().rearrange("(c p) n -> p c n", p=128))
            wkv_sb = wp.tile([128, DC, 2 * DH], FP8)
            nc.scalar.dma_start(wkv_sb[:], wkv_d.ap().rearrange("(c p) n -> p c n", p=128))
            wout_sb = wp.tile([128, INNER // 128, D], BF16)
            nc.scalar.dma_start(wout_sb[:], wout_d.ap().rearrange("(c p) n -> p c n", p=128))

            xnT = rp.tile([128, DC, NS], BF16)       # LN(x)^T   [d, i]
            qT8 = rp.tile([32, 2, H, NS], FP8)       # 16*q^T packed (d=32u+p)
            kT8 = rp.tile([32, 2, J], FP8)           # 16*k^T packed (d=32u+p)
            v8 = rp.tile([128, 2, JT // 2, 68], FP8)  # 16*v rows + 16-col
            oT = rp.tile([128, INNER // 128, NS], BF16)  # attn-out^T [inner, i]
            pT = rp.tile([128, FT, NS], BF16)        # (a*gate)^T [ff, i]

            nc.gpsimd.memset(v8[:, :, :, DH:DH + 1], KV_SCALE)

            def ff1_tile(t, ps_ff):
                wa = ws.tile([128, DC, 128], BF16, tag="wa")
                nc.gpsimd.dma_start(wa[:], wff1_d.ap()[t])
                wg = ws.tile([128, DC, 128], BF16, tag="wg")
                nc.gpsimd.dma_start(wg[:], wff1_d.ap()[t + FT])
                ha = ps_ff.tile([128, NS], F32, tag="ha")
                hg = ps_ff.tile([128, NS], F32, tag="hg")
                for c in range(DC):
                    nc.tensor.matmul(ha[:], wa[:, c, :], xnT[:, c, :],
                                     start=(c == 0), stop=(c == DC - 1))
                for c in range(DC):
                    nc.tensor.matmul(hg[:], wg[:, c, :], xnT[:, c, :],
                                     start=(c == 0), stop=(c == DC - 1))
                ha_sb = work.tile([128, NS], BF16, tag="ha_sb")
                nc.vector.tensor_copy(ha_sb[:], ha[:])
                nc.vector.tensor_mul(pT[:, t, :], ha_sb[:], hg[:])

            def body():
                # ---- phase 1: LN(x shard) + transpose -> xnT ----
                with tc.tile_pool(name="ps_tp", bufs=2,
                                  space=bass.MemorySpace.PSUM) as ps_tp:
                    for it in range(NT):
                        xt = work.tile([128, D], BF16, tag="xt")
                        nc.sync.dma_start(xt[:], xs_d.ap()[ts(it, 128), :])
                        r, nmr = _ln_stats(nc, small, xt, eps_t[:])
                        xn = work.tile([128, D], BF16, tag="xn")
                        nc.vector.tensor_scalar(xn[:], xt[:], r[:], nmr[:],
                                                op0=mybir.AluOpType.mult,
                                                op1=mybir.AluOpType.add)
                        tp = ps_tp.tile([128, DC, 128], BF16, tag="tp")
                        for c in range(DC):
                            nc.tensor.transpose(tp[:, c, :], xn[:, ts(c, 128)], ident[:])
                        nc.vector.tensor_copy(xnT[:, :, ts(it, 128)], tp[:])

                    # ---- phase 2: q projection -> qT8 (fp8 DoubleRow pack) ----
                    with tc.tile_pool(name="ps_q", bufs=2,
                                      space=bass.MemorySpace.PSUM) as ps_q:
                        for h in range(H):
                            qp = ps_q.tile([64, NS], F32, tag="qp")
                            for c in range(DC):
                                nc.tensor.matmul(qp[:], wq_sb[:, c, ts(h, 64)],
                                                 xnT[:, c, :],
                                                 start=(c == 0), stop=(c == DC - 1))
                            qp8 = work.tile([64, NS], FP8, tag="qp8")
                            nc.vector.tensor_copy(qp8[:], qp[:])
                            nc.vector.tensor_copy(qT8[:, 0, h, :], qp8[0:32, :])
                            nc.sync.dma_start(qT8[:, 1, h, :], qp8[32:64, :])

                    # ---- phase 3: ctx LN + transpose + k/v (fp8 DR), ff1 mix ----
                    with (
                        tc.tile_pool(name="ps_kv", bufs=2,
                                     space=bass.MemorySpace.PSUM) as ps_kv,
                        tc.tile_pool(name="ps_ff3", bufs=1,
                                     space=bass.MemorySpace.PSUM) as ps_ff3,
                    ):
                        ff1_done = [0]
                        for jt in range(JT):
                            ct = work.tile([128, D], BF16, tag="ct")
                            nc.sync.dma_start(ct[:], ctx_d.ap()[ts(jt, 128), :])
                            r, nmr = _ln_stats(nc, small, ct, eps_t[:])
                            cn = work.tile([128, D], BF16, tag="cn")
                            nc.vector.tensor_scalar(cn[:], ct[:], r[:], nmr[:],
                                                    op0=mybir.AluOpType.mult,
                                                    op1=mybir.AluOpType.add)
                            cT8 = work.tile([128, DC, 128], FP8, tag="cT8")
                            tp = ps_tp.tile([128, DC, 128], BF16, tag="tp")
                            for c in range(DC):
                                nc.tensor.transpose(tp[:, c, :], cn[:, ts(c, 128)], ident[:])
                            nc.scalar.copy(cT8[:], tp[:])
                            kp = ps_kv.tile([64, 128], F32, tag="kp")
                            vp = ps_kv.tile([128, DH], F32, tag="vp")
                            for c2 in range(DC // 2):
                                nc.tensor.matmul(kp[:], wkv_sb[:, 2 * c2:2 * c2 + 2, 0:DH],
                                                 cT8[:, 2 * c2:2 * c2 + 2, :],
                                                 start=(c2 == 0), stop=(c2 == DC // 2 - 1),
                                                 perf_mode=DR)
                            for c2 in range(DC // 2):
                                nc.tensor.matmul(vp[:], cT8[:, 2 * c2:2 * c2 + 2, :],
                                                 wkv_sb[:, 2 * c2:2 * c2 + 2, DH:2 * DH],
                                                 start=(c2 == 0), stop=(c2 == DC // 2 - 1),
                                                 perf_mode=DR)
                            kp8 = work.tile([64, 128], FP8, tag="kp8")
                            nc.vector.tensor_copy(kp8[:], kp[:])
                            nc.vector.tensor_copy(kT8[:, 0, ts(jt, 128)], kp8[0:32, :])
                            nc.sync.dma_start(kT8[:, 1, ts(jt, 128)], kp8[32:64, :])
                            nc.vector.tensor_copy(v8[:, jt % 2, jt // 2, 0:DH], vp[:])
                            if jt % 2 == 1 and ff1_done[0] < FF1_IN_PH3:
                                ff1_tile(ff1_done[0], ps_ff3)
                                ff1_done[0] += 1

                # ---- phase 4: attention per head (fp8 DR), rest of ff1 ----
                with (
                    tc.tile_pool(name="ps_sim", bufs=2,
                                 space=bass.MemorySpace.PSUM) as ps_sim,
                    tc.tile_pool(name="ps_ao", bufs=1,
                                 space=bass.MemorySpace.PSUM) as ps_ao,
                    tc.tile_pool(name="ps_ot", bufs=1,
                                 space=bass.MemorySpace.PSUM) as ps_ot,
                    tc.tile_pool(name="ps_ff", bufs=1,
                                 space=bass.MemorySpace.PSUM) as ps_ff,
                ):
                    ff1_next = FF1_IN_PH3
                    for h in range(H):
                        ao = ps_ao.tile([128, NT, 68], F32, tag="ao")
                        for jp in range(JT // 2):
                            sim = ps_sim.tile([128, 2, NS], F32, tag="sim")
                            for u in range(2):
                                nc.tensor.matmul(sim[:, u, :],
                                                 kT8[:, :, ts(2 * jp + u, 128)],
                                                 qT8[:, :, h, :],
                                                 start=True, stop=True,
                                                 perf_mode=DR)
                            et8 = expp.tile([128, 2, NS], FP8, tag="et8")
                            nc.scalar.activation(et8[:], sim[:],
                                                 mybir.ActivationFunctionType.Exp,
                                                 bias=expb_t[:], scale=EXP_SCALE)
                            for ib in range(NT):
                                nc.tensor.matmul(ao[:, ib, 0:DH + 1],
                                                 et8[:, :, ts(ib, 128)],
                                                 v8[:, :, jp, 0:DH + 1],
                                                 start=(jp == 0 and ib == 0),
                                                 stop=(jp == JT // 2 - 1
                                                       and ib == NT - 1),
                                                 perf_mode=DR)
                        for ib in range(NT):
                            rec = small.tile([128, 1], F32, tag="rec")
                            nc.vector.reciprocal(rec[:], ao[:, ib, DH:DH + 1])
                            ob = small.tile([128, DH], BF16, tag="ob")
                            nc.vector.tensor_scalar_mul(ob[:], ao[:, ib, 0:DH], rec[:])
                            otp = ps_ot.tile([64, 128], BF16, tag="otp")
                            nc.tensor.transpose(otp[:], ob[:], ident[:])
                            nc.vector.tensor_copy(
                                oT[64 * (h % 2):64 * (h % 2) + 64, h // 2, ts(ib, 128)],
                                otp[:])
                        n_ff = (FT - FF1_IN_PH3) * (h + 1) // H + FF1_IN_PH3
                        while ff1_next < n_ff:
                            ff1_tile(ff1_next, ps_ff)
                            ff1_next += 1

                # ---- phase 5: out = oT^T @ Wout + pT^T @ Wff2 ----
                with tc.tile_pool(name="ps_out", bufs=1,
                                  space=bass.MemorySpace.PSUM) as ps_out:
                    op = [[None] * 2 for _ in range(NT)]
                    for ib in range(NT):
                        for fh in range(2):
                            op_t = ps_out.tile([128, 512], F32, tag=f"op{ib}{fh}")
                            op[ib][fh] = op_t
                    for c in range(INNER // 128):
                        for ib in range(NT):
                            for fh in range(2):
                                nc.tensor.matmul(op[ib][fh][:], oT[:, c, ts(ib, 128)],
                                                 wout_sb[:, c, ts(fh, 512)],
                                                 start=(c == 0), stop=False)
                    for t in range(FT):
                        w2 = ws.tile([128, D], BF16, tag="w2")
                        nc.gpsimd.dma_start(w2[:], wff2_d.ap()[ts(t, 128), :])
                        for ib in range(NT):
                            for fh in range(2):
                                nc.tensor.matmul(op[ib][fh][:], pT[:, t, ts(ib, 128)],
                                                 w2[:, ts(fh, 512)],
                                                 start=False, stop=(t == FT - 1))
                    for ib in range(NT):
                        for fh in range(2):
                            ob_sb = work.tile([128, 512], F32, tag="ob_sb")
                            nc.vector.tensor_copy(ob_sb[:], op[ib][fh][:])
                            nc.sync.dma_start(out_d.ap()[ts(ib, 128), ts(fh, 512)],
                                              ob_sb[:])

            if reps == 1:
                body()
            else:
                with tc.For_i(0, reps, 1):
                    body()
    nc.compile()
    return nc


_CACHE = {}


def _get_nc(reps=1):
    if reps not in _CACHE:
        _CACHE[reps] = build(reps)
    return _CACHE[reps]


def _prep_inputs(x, context, gamma, ctx_gamma, Wq, Wkv, Wout, Wff1, Wff2):
    bf = ml_dtypes.bfloat16
    f8 = ml_dtypes.float8_e4m3
    gamma = np.asarray(gamma, np.float32)
    ctx_gamma = np.asarray(ctx_gamma, np.float32)
    scale = 1.0 / np.sqrt(DH)
    wq = (gamma[:, None] * np.asarray(Wq, np.float32) * scale).astype(bf)
    wkv = (ctx_gamma[:, None] * np.asarray(Wkv, np.float32) * KV_SCALE).astype(f8)
    wout = np.asarray(Wout, np.float32).astype(bf)
    wff1 = (gamma[:, None] * np.asarray(Wff1, np.float32)).astype(bf)
    wff1 = wff1.reshape(DC, 128, 2 * FT, 128).transpose(2, 1, 0, 3).copy()
    wff2 = np.asarray(Wff2, np.float32).astype(bf)
    x = np.asarray(x, np.float32)
    context = np.asarray(context, np.float32)
    in_maps = []
    for c in range(N_CORES):
        in_maps.append({
            "xs": np.ascontiguousarray(x[c * NS:(c + 1) * NS]).astype(bf),
            "ctx": context.astype(bf),
            "wq": wq, "wkv": wkv, "wout": wout, "wff1": wff1, "wff2": wff2,
        })
    return in_maps


def kernel(x, context, gamma, ctx_gamma, Wq, Wkv, Wout, Wff1, Wff2, batch=None,
           **_unused):
    from concourse.bass_utils import run_bass_kernel_spmd

    nc = _get_nc(1)
    in_maps = _prep_inputs(x, context, gamma, ctx_gamma, Wq, Wkv, Wout, Wff1, Wff2)
    res = run_bass_kernel_spmd(nc, in_maps, list(range(N_CORES)))
    return np.concatenate([res.results[c]["out"] for c in range(N_CORES)], axis=0)


# revision 22
# speedup vs baseline: 1.0426x; 1.0426x over previous
"""Cross-attention + parallel-FF block on 8 Trainium2 cores (Bass/Tile).

Sharding: rows of x (sequence-parallel). Each core processes 512 of the 4096
query rows: LN, q-projection (all 8 heads), full attention over the shared
(multi-query) K/V, output projection and the full FF for its rows. K/V are
computed from the full context on every core (replicated, no collectives).

Engine budget (cost-model): exp is Activation-only (~137us) so every other
elementwise op is placed on DVE/Pool; the FF matmuls stay bf16 (fp8 would
blow the 2e-2 error budget) while the attention cluster (k/v proj, sim,
attn@v) runs fp8e4m3 with DoubleRow packing — attention output is a
near-uniform average over 4096 context rows, so fp8 noise washes out.
ff1 tiles are interleaved into the ctx phase and the attention phase so the
PE stays fed while DVE (layernorm) and Act (exp) run.

Scales: Wq carries gamma and dh^-0.5; Wkv carries ctx_gamma and a x16 boost
(keeps fp8 weights out of subnormals). sim_psum = (16q)(16k) = 256 sim, so
exp uses scale=1/256; exp output is 8*exp (bias=ln 8) to sit in e4m3 range;
the "ones" column of V holds 16 so numerator (16v * 8exp) / denominator
(16 * 8exp) is exact.
"""

import math

import numpy as np
import ml_dtypes

import concourse.bass as bass
import concourse.tile as tile
from concourse import bacc, mybir
from concourse.bass import ts
from concourse.masks import make_identity

BF16 = mybir.dt.bfloat16
F32 = mybir.dt.float32
FP8 = mybir.dt.float8e4

N_CORES = 8
N = 4096            # query rows (total)
NS = N // N_CORES   # rows per core = 512
D = 1024            # model dim
J = 4096            # context rows
H = 8               # heads
DH = 64             # head dim
INNER = H * DH      # 512
FF = 4096           # ff_inner
EPS = 1e-5

NT = NS // 128      # 4   query-row tiles per core
JT = J // 128       # 32  context-row tiles
DC = D // 128       # 8   feature chunks
FT = FF // 128      # 32  ff tiles (per a/gate half)

KV_SCALE = 16.0     # host-side Wkv multiplier (fp8 subnormal avoidance)
EXP_SCALE = 1.0 / (KV_SCALE * KV_SCALE)  # sim psum carries 16q * 16k
EXP_BIAS = math.log(8.0)                 # et = 8 * exp(sim)
FF1_IN_PH3 = 12     # ff1 tiles interleaved into the ctx phase (rest: attn)

DR = mybir.MatmulPerfMode.DoubleRow


def _ln_stats(nc, pool, x_tile, eps_ap):
    """bn_stats layer norm stats: returns (r, nmr) f32 [128,1] tiles."""
    stats = pool.tile([128, 2, 6], F32, tag="stats")
    mv = pool.tile([128, 2], F32, tag="mv")
    sq = pool.tile([128, 1], F32, tag="sq")
    r = pool.tile([128, 1], F32, tag="r")
    nmr = pool.tile([128, 1], F32, tag="nmr")
    xv = x_tile[:].rearrange("p (a b) -> p a b", b=512)
    nc.vector.bn_stats(stats[:, 0, :], xv[:, 0, :])
    nc.vector.bn_stats(stats[:, 1, :], xv[:, 1, :])
    nc.vector.bn_aggr(mv[:], stats[:])
    nc.scalar.activation(sq[:], mv[:, 1:2], mybir.ActivationFunctionType.Sqrt,
                         bias=eps_ap, scale=1.0)
    nc.vector.reciprocal(r[:], sq[:])
    nc.vector.scalar_tensor_tensor(nmr[:], mv[:, 0:1], -1.0, r[:],
                                   op0=mybir.AluOpType.mult,
                                   op1=mybir.AluOpType.mult)
    return r, nmr


def build(reps=1):
    nc = bacc.Bacc("TRN2", target_bir_lowering=False, debug=False,
                   num_devices=N_CORES)

    xs_d = nc.dram_tensor("xs", [NS, D], BF16, kind="ExternalInput")
    ctx_d = nc.dram_tensor("ctx", [J, D], BF16, kind="ExternalInput")
    wq_d = nc.dram_tensor("wq", [D, INNER], BF16, kind="ExternalInput")
    wkv_d = nc.dram_tensor("wkv", [D, 2 * DH], FP8, kind="ExternalInput")
    wout_d = nc.dram_tensor("wout", [INNER, D], BF16, kind="ExternalInput")
    wff1_d = nc.dram_tensor("wff1", [2 * FT, 128, DC, 128], BF16,
                            kind="ExternalInput")
    wff2_d = nc.dram_tensor("wff2", [FF, D], BF16, kind="ExternalInput")
    out_d = nc.dram_tensor("out", [NS, D], F32, kind="ExternalOutput")

    with tile.TileContext(nc) as tc:
        with (
            tc.tile_pool(name="const", bufs=1) as constp,
            tc.tile_pool(name="weights", bufs=1) as wp,
            tc.tile_pool(name="resident", bufs=1) as rp,
            tc.tile_pool(name="work", bufs=3) as work,
            tc.tile_pool(name="expt", bufs=6) as expp,
            tc.tile_pool(name="small", bufs=8) as small,
            tc.tile_pool(name="wstream", bufs=6) as ws,
        ):
            ident = constp.tile([128, 128], BF16)
            make_identity(nc, ident[:])
            eps_t = constp.tile([128, 1], F32)
            nc.gpsimd.memset(eps_t[:], EPS)
            expb_t = constp.tile([128, 1], F32)
            nc.gpsimd.memset(expb_t[:], EXP_BIAS)

            wq_sb = wp.tile([128, DC, INNER], BF16)
            nc.sync.dma_start(wq_sb[:], wq_d.ap().rearrange("(c p) n -> p c n", p=128))
            wkv_sb = wp.tile([128, DC, 2 * DH], FP8)
            nc.sync.dma_start(wkv_sb[:], wkv_d.ap().rearrange("(c p) n -> p c n", p=128))
            wout_sb = wp.tile([128, INNER // 128, D], BF16)
            nc.sync.dma_start(wout_sb[:], wout_d.ap().rearrange("(c p) n -> p c n", p=128))

            xnT = rp.tile([128, DC, NS], BF16)       # LN(x)^T   [d, i]
            qT8 = rp.tile([32, 2, H, NS], FP8)       # 16*q^T packed (d=32u+p)
            kT8 = rp.tile([32, 2, J], FP8)           # 16*k^T packed (d=32u+p)
            v8 = rp.tile([128, 2, JT // 2, 68], FP8)  # 16*v rows + 16-col
            oT = rp.tile([128, INNER // 128, NS], BF16)  # attn-out^T [inner, i]
            pT = rp.tile([128, FT, NS], BF16)        # (a*gate)^T [ff, i]

            nc.gpsimd.memset(v8[:, :, :, DH:DH + 1], KV_SCALE)

            def ff1_tile(t, ps_ff):
                wa = ws.tile([128, DC, 128], BF16, tag="wa")
                nc.gpsimd.dma_start(wa[:], wff1_d.ap()[t])
                wg = ws.tile([128, DC, 128], BF16, tag="wg")
                nc.gpsimd.dma_start(wg[:], wff1_d.ap()[t + FT])
                ha = ps_ff.tile([128, NS], F32, tag="ha")
                hg = ps_ff.tile([128, NS], F32, tag="hg")
                for c in range(DC):
                    nc.tensor.matmul(ha[:], wa[:, c, :], xnT[:, c, :],
                                     start=(c == 0), stop=(c == DC - 1))
                for c in range(DC):
                    nc.tensor.matmul(hg[:], wg[:, c, :], xnT[:, c, :],
                                     start=(c == 0), stop=(c == DC - 1))
                ha_sb = work.tile([128, NS], BF16, tag="ha_sb")
                nc.vector.tensor_copy(ha_sb[:], ha[:])
                nc.vector.tensor_mul(pT[:, t, :], ha_sb[:], hg[:])

            def body():
                # ---- phase 1: LN(x shard) + transpose -> xnT ----
                with tc.tile_pool(name="ps_tp", bufs=2,
                                  space=bass.MemorySpace.PSUM) as ps_tp:
                    for it in range(NT):
                        xt = work.tile([128, D], BF16, tag="xt")
                        nc.sync.dma_start(xt[:], xs_d.ap()[ts(it, 128), :])
                        r, nmr = _ln_stats(nc, small, xt, eps_t[:])
                        xn = work.tile([128, D], BF16, tag="xn")
                        nc.vector.tensor_scalar(xn[:], xt[:], r[:], nmr[:],
                                                op0=mybir.AluOpType.mult,
                                                op1=mybir.AluOpType.add)
                        tp = ps_tp.tile([128, DC, 128], BF16, tag="tp")
                        for c in range(DC):
                            nc.tensor.transpose(tp[:, c, :], xn[:, ts(c, 128)], ident[:])
                        nc.vector.tensor_copy(xnT[:, :, ts(it, 128)], tp[:])

                    # ---- phase 2: q projection -> qT8 (fp8 DoubleRow pack) ----
                    with tc.tile_pool(name="ps_q", bufs=2,
                                      space=bass.MemorySpace.PSUM) as ps_q:
                        for h in range(H):
                            qp = ps_q.tile([64, NS], F32, tag="qp")
                            for c in range(DC):
                                nc.tensor.matmul(qp[:], wq_sb[:, c, ts(h, 64)],
                                                 xnT[:, c, :],
                                                 start=(c == 0), stop=(c == DC - 1))
                            qp8 = work.tile([64, NS], FP8, tag="qp8")
                            nc.vector.tensor_copy(qp8[:], qp[:])
                            nc.vector.tensor_copy(qT8[:, 0, h, :], qp8[0:32, :])
                            nc.sync.dma_start(qT8[:, 1, h, :], qp8[32:64, :])

                    # ---- phase 3: ctx LN + transpose + k/v (fp8 DR), ff1 mix ----
                    with (
                        tc.tile_pool(name="ps_kv", bufs=2,
                                     space=bass.MemorySpace.PSUM) as ps_kv,
                        tc.tile_pool(name="ps_ff3", bufs=1,
                                     space=bass.MemorySpace.PSUM) as ps_ff3,
                    ):
                        ff1_done = [0]
                        for jt in range(JT):
                            ct = work.tile([128, D], BF16, tag="ct")
                            nc.sync.dma_start(ct[:], ctx_d.ap()[ts(jt, 128), :])
                            r, nmr = _ln_stats(nc, small, ct, eps_t[:])
                            cn = work.tile([128, D], BF16, tag="cn")
                            nc.vector.tensor_scalar(cn[:], ct[:], r[:], nmr[:],
                                                    op0=mybir.AluOpType.mult,
                                                    op1=mybir.AluOpType.add)
                            cT8 = work.tile([128, DC, 128], FP8, tag="cT8")
                            tp = ps_tp.tile([128, DC, 128], BF16, tag="tp")
                            for c in range(DC):
                                nc.tensor.transpose(tp[:, c, :], cn[:, ts(c, 128)], ident[:])
                            nc.scalar.copy(cT8[:], tp[:])
                            kp = ps_kv.tile([64, 128], F32, tag="kp")
                            vp = ps_kv.tile([128, DH], F32, tag="vp")
                            for c2 in range(DC // 2):
                                nc.tensor.matmul(kp[:], wkv_sb[:, 2 * c2:2 * c2 + 2, 0:DH],
                                                 cT8[:, 2 * c2:2 * c2 + 2, :],
                                                 start=(c2 == 0), stop=(c2 == DC // 2 - 1),
                                                 perf_mode=DR)
                            for c2 in range(DC // 2):
                                nc.tensor.matmul(vp[:], cT8[:, 2 * c2:2 * c2 + 2, :],
                                                 wkv_sb[:, 2 * c2:2 * c2 + 2, DH:2 * DH],
                                                 start=(c2 == 0), stop=(c2 == DC // 2 - 1),
                                                 perf_mode=DR)
                            kp8 = work.tile([64, 128], FP8, tag="kp8")
                            nc.vector.tensor_copy(kp8[:], kp[:])
                            nc.vector.tensor_copy(kT8[:, 0, ts(jt, 128)], kp8[0:32, :])
                            nc.sync.dma_start(kT8[:, 1, ts(jt, 128)], kp8[32:64, :])
                            nc.vector.tensor_copy(v8[:, jt % 2, jt // 2, 0:DH], vp[:])
                            if jt % 2 == 1 and ff1_done[0] < FF1_IN_PH3:
                                ff1_tile(ff1_done[0], ps_ff3)
                                ff1_done[0] += 1

                # ---- phase 4: attention per head (fp8 DR), rest of ff1 ----
                with (
                    tc.tile_pool(name="ps_sim", bufs=2,
                                 space=bass.MemorySpace.PSUM) as ps_sim,
                    tc.tile_pool(name="ps_ao", bufs=1,
                                 space=bass.MemorySpace.PSUM) as ps_ao,
                    tc.tile_pool(name="ps_ot", bufs=1,
                                 space=bass.MemorySpace.PSUM) as ps_ot,
                    tc.tile_pool(name="ps_ff", bufs=1,
                                 space=bass.MemorySpace.PSUM) as ps_ff,
                ):
                    ff1_next = FF1_IN_PH3
                    for h in range(H):
                        ao = ps_ao.tile([128, NT, 68], F32, tag="ao")
                        for jp in range(JT // 2):
                            sim = ps_sim.tile([128, 2, NS], F32, tag="sim")
                            for u in range(2):
                                nc.tensor.matmul(sim[:, u, :],
                                                 kT8[:, :, ts(2 * jp + u, 128)],
                                                 qT8[:, :, h, :],
                                                 start=True, stop=True,
                                                 perf_mode=DR)
                            et8 = expp.tile([128, 2, NS], FP8, tag="et8")
                            nc.scalar.activation(et8[:], sim[:],
                                                 mybir.ActivationFunctionType.Exp,
                                                 bias=expb_t[:], scale=EXP_SCALE)
                            for ib in range(NT):
                                nc.tensor.matmul(ao[:, ib, 0:DH + 1],
                                                 et8[:, :, ts(ib, 128)],
                                                 v8[:, :, jp, 0:DH + 1],
                                                 start=(jp == 0 and ib == 0),
                                                 stop=(jp == JT // 2 - 1
                                                       and ib == NT - 1),
                                                 perf_mode=DR)
                        for ib in range(NT):
                            rec = small.tile([128, 1], F32, tag="rec")
                            nc.vector.reciprocal(rec[:], ao[:, ib, DH:DH + 1])
                            ob = small.tile([128, DH], BF16, tag="ob")
                            nc.vector.tensor_scalar_mul(ob[:], ao[:, ib, 0:DH], rec[:])
                            otp = ps_ot.tile([64, 128], BF16, tag="otp")
                            nc.tensor.transpose(otp[:], ob[:], ident[:])
                            nc.vector.tensor_copy(
                                oT[64 * (h % 2):64 * (h % 2) + 64, h // 2, ts(ib, 128)],
                                otp[:])
                        n_ff = (FT - FF1_IN_PH3) * (h + 1) // H + FF1_IN_PH3
                        while ff1_next < n_ff:
                            ff1_tile(ff1_next, ps_ff)
                            ff1_next += 1

                # ---- phase 5: out = oT^T @ Wout + pT^T @ Wff2 ----
                with tc.tile_pool(name="ps_out", bufs=1,
                                  space=bass.MemorySpace.PSUM) as ps_out:
                    op = [[None] * 2 for _ in range(NT)]
                    for ib in range(NT):
                        for fh in range(2):
                            op_t = ps_out.tile([128, 512], F32, tag=f"op{ib}{fh}")
                            op[ib][fh] = op_t
                    for c in range(INNER // 128):
                        for ib in range(NT):
                            for fh in range(2):
                                nc.tensor.matmul(op[ib][fh][:], oT[:, c, ts(ib, 128)],
                                                 wout_sb[:, c, ts(fh, 512)],
                                                 start=(c == 0), stop=False)
                    for t in range(FT):
                        w2 = ws.tile([128, D], BF16, tag="w2")
                        nc.gpsimd.dma_start(w2[:], wff2_d.ap()[ts(t, 128), :])
                        for ib in range(NT):
                            for fh in range(2):
                                nc.tensor.matmul(op[ib][fh][:], pT[:, t, ts(ib, 128)],
                                                 w2[:, ts(fh, 512)],
                                                 start=False, stop=(t == FT - 1))
                    for ib in range(NT):
                        for fh in range(2):
                            ob_sb = work.tile([128, 512], F32, tag="ob_sb")
                            nc.vector.tensor_copy(ob_sb[:], op[ib][fh][:])
                            nc.sync.dma_start(out_d.ap()[ts(ib, 128), ts(fh, 512)],
                                              ob_sb[:])

            if reps == 1:
                body()
            else:
                with tc.For_i(0, reps, 1):
                    body()
    nc.compile()
    return nc


_CACHE = {}


def _get_nc(reps=1):
    if reps not in _CACHE:
        _CACHE[reps] = build(reps)
    return _CACHE[reps]


def _prep_inputs(x, context, gamma, ctx_gamma, Wq, Wkv, Wout, Wff1, Wff2):
    bf = ml_dtypes.bfloat16
    f8 = ml_dtypes.float8_e4m3
    gamma = np.asarray(gamma, np.float32)
    ctx_gamma = np.asarray(ctx_gamma, np.float32)
    scale = 1.0 / np.sqrt(DH)
    wq = (gamma[:, None] * np.asarray(Wq, np.float32) * scale).astype(bf)
    wkv = (ctx_gamma[:, None] * np.asarray(Wkv, np.float32) * KV_SCALE).astype(f8)
    wout = np.asarray(Wout, np.float32).astype(bf)
    wff1 = (gamma[:, None] * np.asarray(Wff1, np.float32)).astype(bf)
    wff1 = wff1.reshape(DC, 128, 2 * FT, 128).transpose(2, 1, 0, 3).copy()
    wff2 = np.asarray(Wff2, np.float32).astype(bf)
    x = np.asarray(x, np.float32)
    context = np.asarray(context, np.float32)
    in_maps = []
    for c in range(N_CORES):
        in_maps.append({
            "xs": np.ascontiguousarray(x[c * NS:(c + 1) * NS]).astype(bf),
            "ctx": context.astype(bf),
            "wq": wq, "wkv": wkv, "wout": wout, "wff1": wff1, "wff2": wff2,
        })
    return in_maps


def kernel(x, context, gamma, ctx_gamma, Wq, Wkv, Wout, Wff1, Wff2, batch=None,
           **_unused):
    from concourse.bass_utils import run_bass_kernel_spmd

    nc = _get_nc(1)
    in_maps = _prep_inputs(x, context, gamma, ctx_gamma, Wq, Wkv, Wout, Wff1, Wff2)
    res = run_bass_kernel_spmd(nc, in_maps, list(range(N_CORES)))
    return np.concatenate([res.results[c]["out"] for c in range(N_CORES)], axis=0)
